# revision 1
# baseline (speedup 1.0000x reference)
"""Trainium2 Bass kernel for nn_KernelGraphCalcLayer (GNN message passing).

Computation (per batch b):
    h = relu(node_feats @ weight + bias)            # (N, OUT_DIM)
    h = h.reshape(N, K, DK)
    out[n, k, d] = sum_m adj[k, n, m] * h[m, k, d]  # per-kernel dense aggregation

Sharding: batch dim (64) split across 8 NeuronCores, 8 batches per core.
No cross-device communication.

Per-core dataflow:
  - adj (16MB, bulk of HBM traffic) loads via HWDGE at full rate, declared
    float32r (same bits as fp32): PE is_transpose runs at 1.5 cyc/row instead
    of 2.0, and transposition is pure data movement so it stays bit-exact.
    The PSUM->SBUF copy casts to bf16 (VectorE, 1/3 on ScalarE for balance).
  - x (4MB) + W load via SWDGE cast-DMA to bf16 (SWDGE otherwise idle);
    xT comes from hardware DMA-transpose (xbar, 2-byte) on the Scalar HWDGE
    queue -- no PE or VectorE involvement.
  - Linear: psum_h[128,512] = ones.T @ bias (K=1 seed) + sum_i xT_i.T @ W_i,
    ScalarE relu -> h bf16.
  - Aggregation: per node-chunk a full PSUM bank [128,512] accumulates all
    8 kernel slots (2 matmuls each); a single VectorE copy drains it.
"""

import numpy as np

import concourse.bass as bass
import concourse.mybir as mybir
from concourse import bacc
import concourse.tile as tile
from concourse.bass_utils import run_bass_kernel_spmd
from concourse.masks import make_identity

B, N, IN_DIM, OUT_DIM, K = 64, 256, 512, 512, 8
DK = OUT_DIM // K
N_CORES = 8
BPC = B // N_CORES  # batches per core

FP32 = mybir.dt.float32
FP32R = mybir.dt.float32r
CDT = mybir.dt.bfloat16  # compute dtype for matmul operands
P = 128  # SBUF partitions

_compiled = {}


def _build(cdt=CDT):
    nc = bacc.Bacc("TRN2", target_bir_lowering=False, debug=False)
    x_ap = nc.dram_tensor("node_feats", [BPC, N, IN_DIM], FP32, kind="ExternalInput").ap()
    adj_ap = nc.dram_tensor("adj", [BPC, K, N, N], FP32R, kind="ExternalInput").ap()
    w_ap = nc.dram_tensor("weight", [IN_DIM, OUT_DIM], FP32, kind="ExternalInput").ap()
    b_ap = nc.dram_tensor("bias", [OUT_DIM], FP32, kind="ExternalInput").ap()
    out_ap = nc.dram_tensor("out", [BPC, N, OUT_DIM], FP32, kind="ExternalOutput").ap()

    NC2 = N // P       # 2 node chunks of 128
    IC4 = IN_DIM // P  # 4 input-feature chunks

    with tile.TileContext(nc) as tc:
        with (
            tc.tile_pool(name="singles", bufs=1) as singles,
            tc.tile_pool(name="p_x", bufs=3) as p_x,
            tc.tile_pool(name="p_xt", bufs=8) as p_xt,
            tc.tile_pool(name="p_h", bufs=4) as p_h,
            tc.tile_pool(name="p_adj", bufs=12) as p_adj,
            tc.tile_pool(name="p_adjt", bufs=8) as p_adjt,
            tc.tile_pool(name="p_out", bufs=4) as p_out,
            tc.tile_pool(name="ps_ta", bufs=4, space=bass.MemorySpace.PSUM) as ps_ta,
            tc.tile_pool(name="ps_h", bufs=2, space=bass.MemorySpace.PSUM) as ps_h,
            tc.tile_pool(name="ps_o", bufs=2, space=bass.MemorySpace.PSUM) as ps_o,
        ):
            # --- constants ---
            id_src = singles.tile([P, P], FP32)
            make_identity(nc, id_src[:])
            id_f = singles.tile([P, P], FP32R)    # identity for fp32r transposes
            nc.vector.tensor_copy(id_f[:], id_src[:])
            id_c = singles.tile([P, P], cdt)      # identity for bf16 transposes
            make_identity(nc, id_c[:])
            ones_row = singles.tile([1, P], cdt)
            nc.gpsimd.memset(ones_row[:], 1.0)
            bias_c = singles.tile([1, OUT_DIM], cdt)
            nc.gpsimd.dma_start(out=bias_c[:], in_=b_ap[None, :])
            w_sb = [singles.tile([P, OUT_DIM], cdt, name=f"w{ic}")
                    for ic in range(IC4)]

            # DRAM views packing the leading 256 rows into [128, 2, cols]
            x_v = x_ap.rearrange("b (c p) i -> b p c i", p=P)      # [BPC,128,2,512]
            adj_v = adj_ap.rearrange("b k (c p) m -> b k p c m", p=P)

            cast_rr = 0  # round-robin DVE/ACT for adjT casts

            for b in range(BPC):
                # --- prefetch adj[b, k] fp32r via HWDGE, packed [128, 2*256] ---
                a_sbs = []
                for k in range(K):
                    a_sb = p_adj.tile([P, NC2 * N], FP32R, tag="adj",
                                      name=f"a{b}_{k}")
                    nc.sync.dma_start(out=a_sb[:], in_=adj_v[b, k])
                    a_sbs.append(a_sb)

                # --- x: SWDGE cast load bf16, packed [128, 2*512] ---
                x_sb = p_x.tile([P, NC2 * IN_DIM], cdt, tag="x", name=f"x{b}")
                nc.gpsimd.dma_start(out=x_sb[:], in_=x_v[b])

                # --- transpose x -> xT packed by node-chunk (bf16) ---
                # all 4 ic blocks for one nch share a PSUM bank, so each
                # linear half unblocks after 4 transposes instead of 8
                xTn = []
                for nch in range(NC2):
                    t = p_xt.tile([P, IC4 * P], cdt, tag="xT",
                                  name=f"xT{b}_{nch}")
                    pt = ps_ta.tile([P, IC4 * P], cdt, tag="pstf",
                                    name=f"ptx{b}_{nch}")
                    for ic in range(IC4):
                        nc.tensor.transpose(
                            pt[:, ic * P:(ic + 1) * P],
                            x_sb[:, nch * IN_DIM + ic * P:
                                 nch * IN_DIM + (ic + 1) * P],
                            id_c[:])
                    nc.vector.tensor_copy(t[:], pt[:])
                    xTn.append(t)

                if b == 0:
                    # W loads issue after batch 0's x is in flight: the
                    # linear is the first consumer, ~15us into the kernel
                    for ic in range(IC4):
                        nc.gpsimd.dma_start(
                            out=w_sb[ic][:], in_=w_ap[ic * P:(ic + 1) * P, :])

                def xT_sl(ic, nch):
                    return xTn[nch][:, ic * P:(ic + 1) * P]

                # --- linear + bias + relu -> h bf16 [128(n), 512(o)] x2 ---
                h_sb = []
                for nch in range(NC2):
                    ph = ps_h.tile([P, OUT_DIM], FP32, tag="psh", name=f"ph{b}_{nch}")
                    nc.tensor.matmul(ph[:], ones_row[:], bias_c[:],
                                     start=True, stop=False)
                    for ic in range(IC4):
                        nc.tensor.matmul(
                            ph[:], xT_sl(ic, nch), w_sb[ic][:],
                            start=False, stop=(ic == IC4 - 1))
                    ht = p_h.tile([P, OUT_DIM], cdt, tag="h", name=f"h{b}_{nch}")
                    nc.scalar.activation(ht[:], ph[:],
                                         mybir.ActivationFunctionType.Relu)
                    h_sb.append(ht)

                # --- per-kernel aggregation ---
                # full-bank accumulators: all 8 kernel slots land in one bank
                po = [ps_o.tile([P, OUT_DIM], FP32, tag="pso", name=f"po{b}_{i}")
                      for i in range(NC2)]
                for k in range(K):
                    a_sb = a_sbs[k]
                    # transpose -> adjT packed [128(m), 2mch x 256(n)] bf16;
                    # all 4 transposes share one PSUM bank, one cast drains it
                    aT = p_adjt.tile([P, 2 * N], cdt, tag="adjT",
                                     name=f"aT{b}_{k}")
                    pt = ps_ta.tile([P, 2 * N], FP32R, tag="pstf",
                                    name=f"pta{b}_{k}")
                    for mch in range(NC2):
                        for nch in range(NC2):
                            nc.tensor.transpose(
                                pt[:, mch * N + nch * P:
                                   mch * N + (nch + 1) * P],
                                a_sb[:, nch * N + mch * P:
                                     nch * N + (mch + 1) * P],
                                id_f[:])
                    if cast_rr % 4 == 3:
                        nc.scalar.copy(aT[:], pt[:])
                    else:
                        nc.vector.tensor_copy(aT[:], pt[:])
                    cast_rr += 1
                    # po[n, k*DK:+DK] = sum_m adjT[m,n].T @ h[m, k*DK:+DK]
                    for nch in range(NC2):
                        for mch in range(NC2):
                            nc.tensor.matmul(
                                po[nch][:, k * DK:(k + 1) * DK],
                                aT[:, mch * N + nch * P:
                                   mch * N + (nch + 1) * P],
                                h_sb[mch][:, k * DK:(k + 1) * DK],
                                start=(mch == 0), stop=(mch == NC2 - 1))

                # --- drain accumulators + store ---
                for nch in range(NC2):
                    ot = p_out.tile([P, OUT_DIM], FP32, tag="o", name=f"o{b}_{nch}")
                    nc.vector.tensor_copy(ot[:], po[nch][:])
                    nc.scalar.dma_start(
                        out=out_ap[b, nch * P:(nch + 1) * P, :], in_=ot[:])

    nc.compile()
    return nc


def _get_nc():
    if "nc" not in _compiled:
        _compiled["nc"] = _build()
    return _compiled["nc"]


def _run(inputs, trace=False, trace_cores=None):
    nc = _get_nc()
    node_feats = np.ascontiguousarray(inputs["node_feats"], dtype=np.float32)
    adj = np.ascontiguousarray(inputs["adj"], dtype=np.float32)
    weight = np.ascontiguousarray(inputs["weight"], dtype=np.float32)
    bias = np.ascontiguousarray(inputs["bias"], dtype=np.float32)
    in_maps = []
    for c in range(N_CORES):
        sl = slice(c * BPC, (c + 1) * BPC)
        in_maps.append({
            "node_feats": node_feats[sl],
            "adj": adj[sl],
            "weight": weight,
            "bias": bias,
        })
    res = run_bass_kernel_spmd(
        nc, in_maps, core_ids=list(range(N_CORES)),
        trace=trace, trace_cores=trace_cores)
    out = np.concatenate([res.results[c]["out"] for c in range(N_CORES)], axis=0)
    return out.reshape(B, N, OUT_DIM), res


def kernel(**inputs) -> np.ndarray:
    return _run(inputs, trace=False)[0]



# revision 2
# speedup vs baseline: 1.0041x; 1.0041x over previous
"""Trainium2 Bass kernel for nn_KernelGraphCalcLayer (GNN message passing).

Computation (per batch b):
    h = relu(node_feats @ weight + bias)            # (N, OUT_DIM)
    h = h.reshape(N, K, DK)
    out[n, k, d] = sum_m adj[k, n, m] * h[m, k, d]  # per-kernel dense aggregation

Sharding: batch dim (64) split across 8 NeuronCores, 8 batches per core.
No cross-device communication.

v2 dataflow (HBM floor ~73us/core: adj 16.8MB + x 4.2MB + W 1MB reads,
out 4.2MB writes at 358 GB/s):
  - adj loads via SWDGE cast fp32->bf16, (p c) row-pair packing so every
    descriptor covers 2 contiguous HBM rows (2KB): halves descriptor
    emission vs 1KB chunks, halves SBUF write traffic, and bf16 weights
    let every PE matmul/transpose use FWL (fast weight load) which fp32r
    blocks. Two 4-kernel chunks per batch, ~6 batches prefetch depth so
    the DMA stream never waits on compute.
  - x/W/bias also SWDGE cast to bf16 (natural (c p) node chunks so h
    rows line up with adjT's contraction order).
  - PE per batch: 8 xT transposes, 32 adjT transposes (bf16, 1 cyc/row),
    2 bias-seed + 8 linear matmuls, 32 aggregation matmuls.
  - Drains: DVE takes bf16 transpose drains (2x 16-bit rate), ACT takes
    relu + out copies. Stores go on the otherwise idle Sync HWDGE queue
    from a (p c)-packed out tile -> 4KB/partition descriptors.
  - Aggregation of batch b is emitted after the transposes of batch b+1
    so the PE never idles waiting on the relu/drain of its own batch.
"""

import numpy as np

import concourse.bass as bass
import concourse.mybir as mybir
from concourse import bacc
import concourse.tile as tile
from concourse.bass_utils import run_bass_kernel_spmd
from concourse.masks import make_identity

B, N, IN_DIM, OUT_DIM, K = 64, 256, 512, 512, 8
DK = OUT_DIM // K
N_CORES = 8
BPC = B // N_CORES  # batches per core

FP32 = mybir.dt.float32
CDT = mybir.dt.bfloat16
P = 128

_compiled = {}


def _build(cdt=CDT):
    nc = bacc.Bacc("TRN2", target_bir_lowering=False, debug=False)
    x_ap = nc.dram_tensor("node_feats", [BPC, N, IN_DIM], FP32, kind="ExternalInput").ap()
    adj_ap = nc.dram_tensor("adj", [BPC, K, N, N], FP32, kind="ExternalInput").ap()
    w_ap = nc.dram_tensor("weight", [IN_DIM, OUT_DIM], FP32, kind="ExternalInput").ap()
    b_ap = nc.dram_tensor("bias", [OUT_DIM], FP32, kind="ExternalInput").ap()
    out_ap = nc.dram_tensor("out", [BPC, N, OUT_DIM], FP32, kind="ExternalOutput").ap()

    NC2 = N // P       # 2 node chunks of 128
    IC4 = IN_DIM // P  # 4 input-feature chunks
    KH = K // 2        # kernels per adj half-load

    # DRAM views
    # adj: partition p holds rows {2p, 2p+1} (c in {0,1}) -> 2KB contiguous
    # descriptors per (k); free order (k, c, m)
    adj_v = adj_ap.rearrange("b k (p c) m -> b p k c m", c=2)
    # x: natural node chunks (c p): partition p of chunk c = node 128c+p,
    # one 2KB descriptor per (partition, chunk)
    x_v = x_ap.rearrange("b (c p) i -> b p c i", p=P)
    # out: partition p holds rows {2p, 2p+1} -> 4KB contiguous per partition
    out_v = out_ap.rearrange("b (p c) o -> b p c o", c=2)

    with tile.TileContext(nc) as tc:
        with (
            tc.tile_pool(name="singles", bufs=1) as singles,
            tc.tile_pool(name="p_adj", bufs=12) as p_adj,
            tc.tile_pool(name="p_x", bufs=4) as p_x,
            tc.tile_pool(name="p_xt", bufs=4) as p_xt,
            tc.tile_pool(name="p_h", bufs=6) as p_h,
            tc.tile_pool(name="p_at", bufs=20) as p_at,
            tc.tile_pool(name="p_out", bufs=4) as p_out,
            tc.tile_pool(name="ps_t", bufs=4, space=bass.MemorySpace.PSUM) as ps_t,
            tc.tile_pool(name="ps_h", bufs=2, space=bass.MemorySpace.PSUM) as ps_h,
            tc.tile_pool(name="ps_o", bufs=2, space=bass.MemorySpace.PSUM) as ps_o,
        ):
            # --- constants ---
            id_c = singles.tile([P, P], cdt)
            make_identity(nc, id_c[:])
            ones_row = singles.tile([1, P], cdt)
            nc.gpsimd.memset(ones_row[:], 1.0)
            bias_c = singles.tile([1, OUT_DIM], cdt)
            w_sb = [singles.tile([P, OUT_DIM], cdt, name=f"w{ic}")
                    for ic in range(IC4)]

            # --- issue every input DMA up front (single SWDGE FIFO) ---
            # order: x0, adjA0, W+bias, adjB0, x1, adjA1, adjB1, ...
            x_sbs, adj_sbs = [], []
            for b in range(BPC):
                xt = p_x.tile([P, NC2 * IN_DIM], cdt, tag="x", name=f"x{b}")
                nc.gpsimd.dma_start(out=xt[:], in_=x_v[b])
                x_sbs.append(xt)
                halves = []
                for hf in range(2):
                    at = p_adj.tile([P, KH * 2 * N], cdt, tag="adj",
                                    name=f"a{b}_{hf}")
                    nc.gpsimd.dma_start(
                        out=at[:], in_=adj_v[b, :, hf * KH:(hf + 1) * KH])
                    halves.append(at)
                    if b == 0 and hf == 0:
                        for ic in range(IC4):
                            nc.gpsimd.dma_start(
                                out=w_sb[ic][:],
                                in_=w_ap[ic * P:(ic + 1) * P, :])
                        nc.gpsimd.dma_start(out=bias_c[:], in_=b_ap[None, :])
                adj_sbs.append(halves)

            # deferred per-batch state for the software pipeline
            pend = [None] * BPC  # (h_sb, aT_k list)

            def emit_front(b):
                """transposes + linear for batch b"""
                x_sb = x_sbs[b]
                # xT: per node chunk c, 4 ic blocks -> one [128, 512] psum
                xt_sb = []
                for c in range(NC2):
                    pt = ps_t.tile([P, IC4 * P], cdt, tag="pst",
                                   name=f"ptx{b}_{c}")
                    for ic in range(IC4):
                        nc.tensor.transpose(
                            pt[:, ic * P:(ic + 1) * P],
                            x_sb[:, c * IN_DIM + ic * P:
                                 c * IN_DIM + (ic + 1) * P],
                            id_c[:])
                    t = p_xt.tile([P, IC4 * P], cdt, tag="xT",
                                  name=f"xT{b}_{c}")
                    nc.vector.tensor_copy(t[:], pt[:])
                    xt_sb.append(t)

                # adjT: per k, 4 blocks (c, mch) -> one [128, 512] psum
                aT = []
                for k in range(K):
                    a_sb = adj_sbs[b][k // KH]
                    kk = k % KH
                    pt = ps_t.tile([P, 2 * 2 * P], cdt, tag="pst",
                                   name=f"pta{b}_{k}")
                    for c in range(2):
                        for mch in range(NC2):
                            nc.tensor.transpose(
                                pt[:, (c * NC2 + mch) * P:
                                   (c * NC2 + mch + 1) * P],
                                a_sb[:, kk * 2 * N + c * N + mch * P:
                                     kk * 2 * N + c * N + (mch + 1) * P],
                                id_c[:])
                    t = p_at.tile([P, 2 * 2 * P], cdt, tag="aT",
                                  name=f"aT{b}_{k}")
                    nc.vector.tensor_copy(t[:], pt[:])
                    aT.append(t)

                # linear + bias (+relu on ACT) -> h bf16, natural node chunks
                h_sb = []
                for c in range(NC2):
                    ph = ps_h.tile([P, OUT_DIM], FP32, tag="psh",
                                   name=f"ph{b}_{c}")
                    nc.tensor.matmul(ph[:], ones_row[:], bias_c[:],
                                     start=True, stop=False)
                    for ic in range(IC4):
                        nc.tensor.matmul(
                            ph[:], xt_sb[c][:, ic * P:(ic + 1) * P],
                            w_sb[ic][:], start=False, stop=(ic == IC4 - 1))
                    ht = p_h.tile([P, OUT_DIM], cdt, tag="h", name=f"h{b}_{c}")
                    nc.scalar.activation(ht[:], ph[:],
                                         mybir.ActivationFunctionType.Relu)
                    h_sb.append(ht)
                pend[b] = (h_sb, aT)

            def emit_agg(b):
                """aggregation + store for batch b"""
                h_sb, aT = pend[b]
                po = [ps_o.tile([P, OUT_DIM], FP32, tag="pso",
                                name=f"po{b}_{c}") for c in range(2)]
                for k in range(K):
                    for c in range(2):
                        for mch in range(NC2):
                            nc.tensor.matmul(
                                po[c][:, k * DK:(k + 1) * DK],
                                aT[k][:, (c * NC2 + mch) * P:
                                      (c * NC2 + mch + 1) * P],
                                h_sb[mch][:, k * DK:(k + 1) * DK],
                                start=(mch == 0), stop=(mch == NC2 - 1))
                ot = p_out.tile([P, 2 * OUT_DIM], FP32, tag="o", name=f"o{b}")
                for c in range(2):
                    nc.scalar.copy(ot[:, c * OUT_DIM:(c + 1) * OUT_DIM],
                                   po[c][:])
                nc.sync.dma_start(out=out_v[b], in_=ot[:])

            # software pipeline: agg(b) is emitted after front(b+1) so the
            # PE never waits on batch b's relu/drains
            emit_front(0)
            for b in range(1, BPC):
                emit_front(b)
                emit_agg(b - 1)
            emit_agg(BPC - 1)

    nc.compile()
    return nc


def _get_nc():
    if "nc" not in _compiled:
        _compiled["nc"] = _build()
    return _compiled["nc"]


def _run(inputs, trace=False, trace_cores=None):
    nc = _get_nc()
    node_feats = np.ascontiguousarray(inputs["node_feats"], dtype=np.float32)
    adj = np.ascontiguousarray(inputs["adj"], dtype=np.float32)
    weight = np.ascontiguousarray(inputs["weight"], dtype=np.float32)
    bias = np.ascontiguousarray(inputs["bias"], dtype=np.float32)
    in_maps = []
    for c in range(N_CORES):
        sl = slice(c * BPC, (c + 1) * BPC)
        in_maps.append({
            "node_feats": node_feats[sl],
            "adj": adj[sl],
            "weight": weight,
            "bias": bias,
        })
    res = run_bass_kernel_spmd(
        nc, in_maps, core_ids=list(range(N_CORES)),
        trace=trace, trace_cores=trace_cores)
    out = np.concatenate([res.results[c]["out"] for c in range(N_CORES)], axis=0)
    return out.reshape(B, N, OUT_DIM), res


def kernel(**inputs) -> np.ndarray:
    return _run(inputs, trace=False)[0]


# revision 10
# speedup vs baseline: 1.0974x; 1.0929x over previous
"""Trainium2 Bass kernel for nn_KernelGraphCalcLayer (GNN message passing).

Computation (per batch b):
    h = relu(node_feats @ weight + bias)            # (N, OUT_DIM)
    h = h.reshape(N, K, DK)
    out[n, k, d] = sum_m adj[k, n, m] * h[m, k, d]  # per-kernel dense aggregation

Sharding: batch dim (64) split across 8 NeuronCores, 8 batches per core.
No cross-device communication.

v3 dataflow (HBM floor ~73us/core: 22.4MB reads + 4.2MB writes @358GB/s):
  - adj is the only SWDGE traffic: cast fp32->bf16 in flight, (p c)
    row-pair packing -> 2KB descriptors, two 4-kernel chunks per batch,
    ~6 batches of prefetch so the stream never waits on compute. The
    GpSimd queue carries nothing ahead of the first adj emission except
    the identity build (slotted between chunk A and B of batch 0).
  - x/W/bias load fp32 on the otherwise-idle Sync HWDGE queue; x is cast
    to bf16 on GpSimd (slack between adj emissions), W/bias on DVE.
  - PE warmup: a burst of back-to-back dummy matmuls on a memset tile
    right after the preamble, so the HAM clock gate lifts (1.2->2.4GHz)
    by the time the first real transposes arrive. v2 ran its first 25us
    at half clock without this.
  - All PE work in bf16 (FWL-eligible): 8 xT + 32 adjT transposes,
    2 bias-seed + 8 linear matmuls, 32 aggregation matmuls per batch.
  - Drains: DVE takes bf16 transpose drains, ACT takes relu + out
    copies. Stores ride Sync behind the loads from a (p c)-packed out
    tile -> 4KB/partition descriptors.
  - agg(b) is emitted after front(b+1) so the PE never waits on its own
    batch's relu/drains.
"""

import numpy as np

import concourse.bass as bass
import concourse.mybir as mybir
from concourse import bacc
import concourse.tile as tile
from concourse.bass_utils import run_bass_kernel_spmd
from concourse.masks import make_identity

B, N, IN_DIM, OUT_DIM, K = 64, 256, 512, 512, 8
DK = OUT_DIM // K
N_CORES = 8
BPC = B // N_CORES  # batches per core

FP32 = mybir.dt.float32
CDT = mybir.dt.bfloat16
P = 128

WARMUP_MM = 18  # ~3.5us of back-to-back dummy matmuls at cold clock

_compiled = {}


def _build(cdt=CDT):
    nc = bacc.Bacc("TRN2", target_bir_lowering=False, debug=False)
    x_ap = nc.dram_tensor("node_feats", [BPC, N, IN_DIM], FP32, kind="ExternalInput").ap()
    adj_ap = nc.dram_tensor("adj", [BPC, K, N, N], FP32, kind="ExternalInput").ap()
    w_ap = nc.dram_tensor("weight", [IN_DIM, OUT_DIM], FP32, kind="ExternalInput").ap()
    b_ap = nc.dram_tensor("bias", [OUT_DIM], FP32, kind="ExternalInput").ap()
    out_ap = nc.dram_tensor("out", [BPC, N, OUT_DIM], FP32, kind="ExternalOutput").ap()

    NC2 = N // P       # 2 node chunks of 128
    IC4 = IN_DIM // P  # 4 input-feature chunks
    KH = K // 2        # kernels per adj half-load

    # adj: partition p holds rows {2p, 2p+1} (c in {0,1}) -> one 2KB
    # contiguous descriptor per (partition, k)
    adj_v = adj_ap.rearrange("b k (p c) m -> b p k c m", c=2)
    # x: natural node chunks (c p): partition p of chunk c = node 128c+p
    x_v = x_ap.rearrange("b (c p) i -> b p c i", p=P)
    # out: partition p holds rows {2p, 2p+1} -> 4KB contiguous per partition
    out_v = out_ap.rearrange("b (p c) o -> b p c o", c=2)

    with tile.TileContext(nc) as tc:
        with (
            tc.tile_pool(name="singles", bufs=1) as singles,
            tc.tile_pool(name="p_adj", bufs=12) as p_adj,
            tc.tile_pool(name="p_xf", bufs=6) as p_xf,
            tc.tile_pool(name="p_x", bufs=4) as p_x,
            tc.tile_pool(name="p_xt", bufs=3) as p_xt,
            tc.tile_pool(name="p_h", bufs=6) as p_h,
            tc.tile_pool(name="p_at", bufs=10) as p_at,
            tc.tile_pool(name="p_out", bufs=4) as p_out,
            tc.tile_pool(name="ps_t", bufs=3, space=bass.MemorySpace.PSUM) as ps_t,
            tc.tile_pool(name="ps_h", bufs=2, space=bass.MemorySpace.PSUM) as ps_h,
            tc.tile_pool(name="ps_o", bufs=2, space=bass.MemorySpace.PSUM) as ps_o,
        ):
            # --- Sync HWDGE: W, bias, then all x (fp32, no cast) ---
            w_f32 = [singles.tile([P, OUT_DIM], FP32, name=f"wf{ic}")
                     for ic in range(IC4)]
            for ic in range(IC4):
                nc.sync.dma_start(out=w_f32[ic][:],
                                  in_=w_ap[ic * P:(ic + 1) * P, :])
            bias_f32 = singles.tile([1, OUT_DIM], FP32)
            nc.sync.dma_start(out=bias_f32[:], in_=b_ap[None, :])
            xf_sbs = []
            for b in range(BPC):
                xf = p_xf.tile([P, NC2 * IN_DIM], FP32, tag="xf", name=f"xf{b}")
                nc.sync.dma_start(out=xf[:], in_=x_v[b])
                xf_sbs.append(xf)

            # --- GpSimd SWDGE: adj halves; identity build slotted after
            # the first chunk so the stream starts immediately ---
            adj_sbs = [[None, None] for _ in range(BPC)]
            id_c = singles.tile([P, P], cdt)

            def load_adj(b, hf, tag_extra=""):
                at = p_adj.tile([P, KH * 2 * N], cdt, tag="adj",
                                name=f"a{b}_{hf}")
                nc.gpsimd.dma_start(
                    out=at[:], in_=adj_v[b, :, hf * KH:(hf + 1) * KH])
                adj_sbs[b][hf] = at

            load_adj(0, 0)
            make_identity(nc, id_c[:])
            load_adj(0, 1)

            # --- DVE: warmup tile + ones row + W/bias casts ---
            warm = singles.tile([P, P], cdt)
            nc.vector.memset(warm[:], 0.125)
            ones_row = singles.tile([1, P], cdt)
            nc.vector.memset(ones_row[:], 1.0)
            w_sb = [singles.tile([P, OUT_DIM], cdt, name=f"w{ic}")
                    for ic in range(IC4)]
            for ic in range(IC4):
                nc.vector.tensor_copy(w_sb[ic][:], w_f32[ic][:])
            bias_c = singles.tile([1, OUT_DIM], cdt)
            nc.vector.tensor_copy(bias_c[:], bias_f32[:])

            # --- PE warmup: back-to-back dummy matmuls lift the HAM gate
            # (borrows the ps_o ring — same tag/shape as the agg psum) ---
            pw = [ps_o.tile([P, OUT_DIM], FP32, tag="pso", name=f"wm{i}")
                  for i in range(2)]
            for i in range(WARMUP_MM):
                nc.tensor.matmul(pw[i % 2][:, :P], warm[:], warm[:],
                                 start=True, stop=True)

            # remaining adj loads + x casts on GpSimd (emission stays well
            # ahead of the SDMA drain rate)
            x_sbs = []
            for b in range(BPC):
                if b > 0:
                    load_adj(b, 0)
                    load_adj(b, 1)
                xt = p_x.tile([P, NC2 * IN_DIM], cdt, tag="x", name=f"x{b}")
                nc.gpsimd.tensor_copy(xt[:], xf_sbs[b][:])
                x_sbs.append(xt)

            pend = [None] * BPC

            def emit_front(b):
                """transposes + linear for batch b"""
                x_sb = x_sbs[b]
                # xT: all 8 blocks (c, ic) share one full-bank psum tile,
                # drained by a single DVE copy
                pt = ps_t.tile([P, NC2 * IC4 * P], cdt, tag="pst",
                               name=f"ptx{b}")
                for c in range(NC2):
                    for ic in range(IC4):
                        nc.tensor.transpose(
                            pt[:, (c * IC4 + ic) * P:(c * IC4 + ic + 1) * P],
                            x_sb[:, c * IN_DIM + ic * P:
                                 c * IN_DIM + (ic + 1) * P],
                            id_c[:])
                xt_sb = p_xt.tile([P, NC2 * IC4 * P], cdt, tag="xT",
                                  name=f"xT{b}")
                nc.vector.tensor_copy(xt_sb[:], pt[:])

                # adjT: one full-bank psum tile per k-pair (8 transposes),
                # one DVE drain each
                aT = []
                for kp in range(K // 2):
                    pt = ps_t.tile([P, 8 * P], cdt, tag="pst",
                                   name=f"pta{b}_{kp}")
                    for kk in range(2):
                        k = kp * 2 + kk
                        a_sb = adj_sbs[b][k // KH]
                        klocal = k % KH
                        for c in range(2):
                            for mch in range(NC2):
                                nc.tensor.transpose(
                                    pt[:, (kk * 4 + c * NC2 + mch) * P:
                                       (kk * 4 + c * NC2 + mch + 1) * P],
                                    a_sb[:, klocal * 2 * N + c * N + mch * P:
                                         klocal * 2 * N + c * N + (mch + 1) * P],
                                    id_c[:])
                    t = p_at.tile([P, 8 * P], cdt, tag="aT",
                                  name=f"aT{b}_{kp}")
                    nc.vector.tensor_copy(t[:], pt[:])
                    aT.append(t)

                h_sb = []
                for c in range(NC2):
                    ph = ps_h.tile([P, OUT_DIM], FP32, tag="psh",
                                   name=f"ph{b}_{c}")
                    nc.tensor.matmul(ph[:], ones_row[:], bias_c[:],
                                     start=True, stop=False)
                    for ic in range(IC4):
                        nc.tensor.matmul(
                            ph[:], xt_sb[:, (c * IC4 + ic) * P:
                                         (c * IC4 + ic + 1) * P],
                            w_sb[ic][:], start=False, stop=(ic == IC4 - 1))
                    ht = p_h.tile([P, OUT_DIM], cdt, tag="h", name=f"h{b}_{c}")
                    nc.scalar.activation(ht[:], ph[:],
                                         mybir.ActivationFunctionType.Relu)
                    h_sb.append(ht)
                pend[b] = (h_sb, aT)

            def emit_agg(b):
                """aggregation + store for batch b"""
                h_sb, aT = pend[b]
                po = [ps_o.tile([P, OUT_DIM], FP32, tag="pso",
                                name=f"po{b}_{c}") for c in range(2)]
                for k in range(K):
                    kp, kk = k // 2, k % 2
                    for c in range(2):
                        for mch in range(NC2):
                            nc.tensor.matmul(
                                po[c][:, k * DK:(k + 1) * DK],
                                aT[kp][:, (kk * 4 + c * NC2 + mch) * P:
                                       (kk * 4 + c * NC2 + mch + 1) * P],
                                h_sb[mch][:, k * DK:(k + 1) * DK],
                                start=(mch == 0), stop=(mch == NC2 - 1))
                ot = p_out.tile([P, 2 * OUT_DIM], FP32, tag="o", name=f"o{b}")
                for c in range(2):
                    nc.scalar.copy(ot[:, c * OUT_DIM:(c + 1) * OUT_DIM],
                                   po[c][:])
                nc.sync.dma_start(out=out_v[b], in_=ot[:])

            emit_front(0)
            for b in range(1, BPC):
                emit_front(b)
                emit_agg(b - 1)
            emit_agg(BPC - 1)

    nc.compile()
    return nc


def _get_nc():
    if "nc" not in _compiled:
        _compiled["nc"] = _build()
    return _compiled["nc"]


def _run(inputs, trace=False, trace_cores=None):
    nc = _get_nc()
    node_feats = np.ascontiguousarray(inputs["node_feats"], dtype=np.float32)
    adj = np.ascontiguousarray(inputs["adj"], dtype=np.float32)
    weight = np.ascontiguousarray(inputs["weight"], dtype=np.float32)
    bias = np.ascontiguousarray(inputs["bias"], dtype=np.float32)
    in_maps = []
    for c in range(N_CORES):
        sl = slice(c * BPC, (c + 1) * BPC)
        in_maps.append({
            "node_feats": node_feats[sl],
            "adj": adj[sl],
            "weight": weight,
            "bias": bias,
        })
    res = run_bass_kernel_spmd(
        nc, in_maps, core_ids=list(range(N_CORES)),
        trace=trace, trace_cores=trace_cores)
    out = np.concatenate([res.results[c]["out"] for c in range(N_CORES)], axis=0)
    return out.reshape(B, N, OUT_DIM), res


def kernel(**inputs) -> np.ndarray:
    return _run(inputs, trace=False)[0]
